# revision 1
# baseline (speedup 1.0000x reference)
"""Distributed Trainium2 kernel for nn_AttentionLayer (B=2, S=2048, D=2048, H=16).

Sharding: core c = (batch b, head-group g) with b = c // 4, g = c % 4.
Each core owns 4 heads (512 of the 2048 projection dims) of one batch element:
  - computes qp/kp/vp for its head-group (bf16 matmuls, f32 accumulation),
  - masked softmax attention for its 4 heads (no max-subtraction: scores are
    bounded; masked entries become exactly 0 via exp(s) * (1-mask)),
  - its partial output projection (Wo row-shard).
The 4 partial outputs per batch are summed on the host (the axon PJRT path
in this container cannot execute cross-core collectives — jax-level psum and
NEFF ReduceScatter both hang — so the device-side RS exists behind USE_RS
but defaults off). Host side otherwise only transposes/casts/concatenates.

Inputs staged per core (host pre-transposed, bf16):
  qT/kT/vT [D, S], maskT [S, S] (=1-mask.T), wqT/wkT/wvT [D, 512], woT [512, D]
Outputs per core (f32): kpT_out [512, S], vp_out [S, 512],
  outp_out [S, D] partial (or out_rs [512, D] when USE_RS).

Phase B is software-pipelined: scores/exp/mask of step (t,h) are issued
before ctx/transpose of step (t,h-1) so the in-order PE always has matmul
work while the ScalarE exp + VectorE mask-mul pipeline fills attn tiles.
"""

import numpy as np
import ml_dtypes

import concourse.bass as bass  # noqa: F401
import concourse.mybir as mybir
import concourse.tile as tile
from concourse import bacc
from concourse import bass_utils
from concourse.masks import make_identity

BF16 = mybir.dt.bfloat16
F32 = mybir.dt.float32
nbf16 = ml_dtypes.bfloat16

B, S, D, H = 2, 2048, 2048, 16
GH = 4                # heads per core
DH = 128              # head dim
GD = GH * DH          # 512 local projection dims
KC = D // 128         # 16 contraction chunks
NB = 4                # query blocks
BLK = S // NB         # 512
N_CORES = 8
SCALE = float(1.0 / np.sqrt(DH))

# Use the on-device ReduceScatter for the Wo partial sums. If False, each
# core DMAs its full partial output and the host sums the 4 partials.
import os as _os
USE_RS = _os.environ.get("ATTN_USE_RS", "0") == "1"

_CACHE = {}


def _build():
    nc = bacc.Bacc(
        "TRN2", target_bir_lowering=False, debug=False, num_devices=N_CORES
    )
    AF = mybir.ActivationFunctionType

    qT = nc.dram_tensor("qT", [D, S], BF16, kind="ExternalInput")
    kT = nc.dram_tensor("kT", [D, S], BF16, kind="ExternalInput")
    vT = nc.dram_tensor("vT", [D, S], BF16, kind="ExternalInput")
    maskT = nc.dram_tensor("maskT", [S, S], BF16, kind="ExternalInput")
    wqT = nc.dram_tensor("wqT", [D, GD], BF16, kind="ExternalInput")
    wkT = nc.dram_tensor("wkT", [D, GD], BF16, kind="ExternalInput")
    wvT = nc.dram_tensor("wvT", [D, GD], BF16, kind="ExternalInput")
    woT = nc.dram_tensor("woT", [GD, D], BF16, kind="ExternalInput")
    kpT_out = nc.dram_tensor("kpT_out", [GD, S], F32, kind="ExternalOutput")
    vp_out = nc.dram_tensor("vp_out", [S, GD], F32, kind="ExternalOutput")
    if USE_RS:
        out_rs = nc.dram_tensor("out_rs", [NB * 128, D], F32, kind="ExternalOutput")
    else:
        # bf16 partials: the host upcasts and sums; halves the drain + DMA cost
        outp_out = nc.dram_tensor("outp_out", [S, D], BF16, kind="ExternalOutput")

    with tile.TileContext(nc) as tc:
        with tc.tile_pool(name="res", bufs=1) as res:
            # ---- resident SBUF tensors (live across both phases) ----
            wo_sb = res.tile([128, GH * D], BF16, name="wo_sb", tag="wo")
            qp_sb = res.tile([128, GH * S], BF16, name="qp_sb", tag="qp")
            kp_sb = res.tile([128, GH * S], BF16, name="kp_sb", tag="kp")
            # vp + per-head ones column: s-chunk sc at cols sc*516, head h at
            # +h*129 (128 vp dims then one 1.0 column for the softmax denom)
            vpo_sb = res.tile([128, KC * 516], BF16, name="vpo_sb", tag="vpo")
            ident = res.tile([128, 128], BF16, name="ident", tag="ident")

            make_identity(nc, ident[:])
            nc.gpsimd.memset(vpo_sb[:], 1.0)

            # ---------------- Phase A: projections ----------------
            with (
                tc.tile_pool(name="wpool", bufs=1) as wpool,
                tc.tile_pool(name="stream", bufs=10) as stream,
                tc.tile_pool(name="stageA", bufs=6) as stageA,
                tc.tile_pool(name="psA", bufs=8, space="PSUM") as psA,
            ):
                wq_sb = wpool.tile([128, KC * GD], BF16, name="wq_sb", tag="wq")
                wk_sb = wpool.tile([128, KC * GD], BF16, name="wk_sb", tag="wk")
                wv_sb = wpool.tile([128, KC * GD], BF16, name="wv_sb", tag="wv")

                # qpT / kpT in [dout, s] orientation. Weight-chunk DMAs are
                # interleaved with the first activation chunks so the first
                # matmul isn't queued behind megabytes of weight traffic.
                # LDWEIGHTS amortization: each stationary weight slice serves
                # the two s-chunks of an n-pair (8 PSUM banks in flight).
                for xdram, wdram, w_sb, dst_bf, f32out in (
                    (qT, wqT, wq_sb, qp_sb, None),
                    (kT, wkT, wk_sb, kp_sb, kpT_out),
                ):
                    for npair in range(2):
                        pss = [
                            psA.tile([128, 512], F32, name=f"psa{j}", tag="psa")
                            for j in range(8)
                        ]
                        for kc in range(KC):
                            if npair == 0:
                                nc.sync.dma_start(
                                    out=w_sb[:, kc * GD:(kc + 1) * GD],
                                    in_=wdram[kc * 128:(kc + 1) * 128, :],
                                )
                            xss = []
                            for n2 in range(2):
                                n = npair * 2 + n2
                                xs = stream.tile([128, 512], BF16, name="xs",
                                                 tag="xs")
                                nc.sync.dma_start(
                                    out=xs[:],
                                    in_=xdram[kc * 128:(kc + 1) * 128,
                                              n * 512:(n + 1) * 512],
                                )
                                xss.append(xs)
                            for m in range(4):
                                for n2 in range(2):
                                    nc.tensor.matmul(
                                        pss[m * 2 + n2][:],
                                        lhsT=w_sb[:, kc * GD + m * 128:
                                                  kc * GD + (m + 1) * 128],
                                        rhs=xss[n2][:],
                                        start=(kc == 0),
                                        stop=(kc == KC - 1),
                                    )
                        for m in range(4):
                            for n2 in range(2):
                                n = npair * 2 + n2
                                ps = pss[m * 2 + n2]
                                nc.scalar.copy(
                                    dst_bf[:, m * S + n * 512:
                                           m * S + (n + 1) * 512],
                                    ps[:],
                                )
                                if f32out is not None:
                                    st = stageA.tile([128, 512], F32, name="st",
                                                     tag="st")
                                    nc.vector.tensor_copy(st[:], ps[:])
                                    nc.sync.dma_start(
                                        out=f32out[m * 128:(m + 1) * 128,
                                                   n * 512:(n + 1) * 512],
                                        in_=st[:],
                                    )
                # vp in [s, dout] orientation
                for kc in range(KC):
                    nc.sync.dma_start(
                        out=wv_sb[:, kc * GD:(kc + 1) * GD],
                        in_=wvT[kc * 128:(kc + 1) * 128, :],
                    )
                for h in range(GH):
                    nc.sync.dma_start(
                        out=wo_sb[:, h * D:(h + 1) * D],
                        in_=woT[h * 128:(h + 1) * 128, :],
                    )
                for mq in range(4):
                    pss = [
                        psA.tile([128, 512], F32, name=f"psa{m}", tag="psa")
                        for m in range(4)
                    ]
                    for kc in range(KC):
                        xs = stream.tile([128, 512], BF16, name="xs", tag="xs")
                        nc.sync.dma_start(
                            out=xs[:],
                            in_=vT[kc * 128:(kc + 1) * 128,
                                   mq * 512:(mq + 1) * 512],
                        )
                        for mi in range(4):
                            nc.tensor.matmul(
                                pss[mi][:],
                                lhsT=xs[:, mi * 128:(mi + 1) * 128],
                                rhs=wv_sb[:, kc * GD:(kc + 1) * GD],
                                start=(kc == 0),
                                stop=(kc == KC - 1),
                            )
                    for mi in range(4):
                        sc = mq * 4 + mi
                        for h in range(GH):
                            nc.scalar.copy(
                                vpo_sb[:, sc * 516 + h * 129:
                                       sc * 516 + h * 129 + 128],
                                pss[mi][:, h * 128:(h + 1) * 128],
                            )
                        st = stageA.tile([128, 512], F32, name="st", tag="st")
                        nc.vector.tensor_copy(st[:], pss[mi][:])
                        nc.sync.dma_start(
                            out=vp_out[sc * 128:(sc + 1) * 128, :], in_=st[:]
                        )

            # ---------------- Phase B: attention + out-proj ----------------
            with (
                tc.tile_pool(name="mp", bufs=24) as mp,
                tc.tile_pool(name="apl", bufs=34) as apl,
                tc.tile_pool(name="cpl", bufs=8) as cpl,
                tc.tile_pool(name="stageB", bufs=6) as stageB,
                tc.tile_pool(name="psS", bufs=4, space="PSUM") as psS,
                tc.tile_pool(name="psCT", bufs=2, space="PSUM") as psCT,
                tc.tile_pool(name="psO", bufs=2, space="PSUM") as psO,
                tc.tile_pool(name="dpool", bufs=4, space="DRAM") as dpool,
            ):
                mask_tiles = {}   # t -> list of 16 tiles
                attn_tiles = {}   # (t, h) -> list of 16 tiles
                ctx_tiles = {}    # t -> list of 4 ct tiles (per head)

                def emit_mask_dmas(t):
                    mts = []
                    for skc in range(KC):
                        mt = mp.tile([128, 512], BF16, name="mt", tag="mt")
                        nc.sync.dma_start(
                            out=mt[:],
                            in_=maskT[skc * 128:(skc + 1) * 128,
                                      t * 512:(t + 1) * 512],
                        )
                        mts.append(mt)
                    mask_tiles[t] = mts

                def emit_scores(t, h):
                    mts = mask_tiles[t]
                    attn = []
                    for skc in range(KC):
                        sps = psS.tile([128, 512], F32, name="sps", tag="sps")
                        nc.tensor.matmul(
                            sps[:],
                            lhsT=kp_sb[:, h * S + skc * 128:
                                       h * S + (skc + 1) * 128],
                            rhs=qp_sb[:, h * S + t * 512: h * S + (t + 1) * 512],
                            start=True,
                            stop=True,
                        )
                        at = apl.tile([128, 512], BF16, name="at", tag="at")
                        nc.scalar.activation(at[:], sps[:], AF.Exp, scale=SCALE)
                        nc.vector.tensor_mul(at[:], at[:], mts[skc][:])
                        attn.append(at)
                    attn_tiles[(t, h)] = attn

                def emit_ctx(t, h):
                    attn = attn_tiles.pop((t, h))
                    ct = ctx_tiles.setdefault(t, {})
                    cth = cpl.tile([128, 512], BF16, name="ct", tag="ct")
                    for mm in range(4):
                        cps = psCT.tile([128, 129], F32, name="cps", tag="psct")
                        for skc in range(KC):
                            nc.tensor.matmul(
                                cps[:],
                                lhsT=attn[skc][:, mm * 128:(mm + 1) * 128],
                                rhs=vpo_sb[:, skc * 516 + h * 129:
                                           skc * 516 + (h + 1) * 129],
                                start=(skc == 0),
                                stop=(skc == KC - 1),
                            )
                        rec = stageB.tile([128, 1], F32, name="rec", tag="rec")
                        nc.vector.reciprocal(rec[:], cps[:, 128:129])
                        ctn = stageB.tile([128, 128], BF16, name="ctn", tag="ctn")
                        nc.vector.tensor_scalar_mul(ctn[:], cps[:, 0:128], rec[:])
                        tps = psCT.tile([128, 128], BF16, name="tps", tag="psct")
                        nc.tensor.transpose(tps[:], ctn[:], ident[:])
                        nc.vector.tensor_copy(cth[:, mm * 128:(mm + 1) * 128],
                                              tps[:])
                    ct[h] = cth

                def emit_outproj(t):
                    cts = ctx_tiles.pop(t)
                    if USE_RS:
                        rs_in = dpool.tile([BLK, D], F32, name="rs_in",
                                           tag="rs_in")
                        rs_out = dpool.tile([128, D], F32, name="rs_out",
                                            tag="rs_out")
                    # LDW amortization: each ct stationary serves an n-pair
                    for mm in range(4):
                        for npair in range(2):
                            ops2 = [psO.tile([128, 512], F32, name=f"ops{j}",
                                             tag="ops") for j in range(2)]
                            for h in range(GH):
                                for n2 in range(2):
                                    n = npair * 2 + n2
                                    nc.tensor.matmul(
                                        ops2[n2][:],
                                        lhsT=cts[h][:, mm * 128:(mm + 1) * 128],
                                        rhs=wo_sb[:, h * D + n * 512:
                                                  h * D + (n + 1) * 512],
                                        start=(h == 0),
                                        stop=(h == GH - 1),
                                    )
                            for n2 in range(2):
                                n = npair * 2 + n2
                                ost = stageB.tile([128, 512],
                                                  F32 if USE_RS else BF16,
                                                  name="ost", tag="ost")
                                nc.vector.tensor_copy(ost[:], ops2[n2][:])
                                if USE_RS:
                                    nc.sync.dma_start(
                                        out=rs_in[mm * 128:(mm + 1) * 128,
                                                  n * 512:(n + 1) * 512],
                                        in_=ost[:],
                                    )
                                else:
                                    nc.sync.dma_start(
                                        out=outp_out[t * BLK + mm * 128:
                                                     t * BLK + (mm + 1) * 128,
                                                     n * 512:(n + 1) * 512],
                                        in_=ost[:],
                                    )
                    if USE_RS:
                        nc.gpsimd.collective_compute(
                            "ReduceScatter",
                            mybir.AluOpType.add,
                            replica_groups=[[0, 1, 2, 3], [4, 5, 6, 7]],
                            ins=[rs_in[:, :].opt()],
                            outs=[rs_out[:, :].opt()],
                        )
                        nc.sync.dma_start(
                            out=out_rs[t * 128:(t + 1) * 128, :], in_=rs_out[:, :]
                        )

                # software pipeline: scores(step i+1) before ctx(step i)
                steps = [(t, h) for t in range(NB) for h in range(GH)]
                emit_mask_dmas(0)
                emit_scores(0, 0)
                for i in range(1, len(steps)):
                    t, h = steps[i]
                    if h == 0:
                        emit_mask_dmas(t)
                    emit_scores(t, h)
                    pt, ph = steps[i - 1]
                    emit_ctx(pt, ph)
                    if ph == GH - 1:
                        emit_outproj(pt)
                emit_ctx(*steps[-1])
                emit_outproj(steps[-1][0])

    nc.compile()
    return nc


def get_nc():
    if "nc" not in _CACHE:
        _CACHE["nc"] = _build()
    return _CACHE["nc"]


def make_in_maps(inputs):
    q = np.asarray(inputs["q"], np.float32)
    k = np.asarray(inputs["k"], np.float32)
    v = np.asarray(inputs["v"], np.float32)
    mask = np.asarray(inputs["mask"])
    Wq = np.asarray(inputs["Wq"], np.float32)
    Wk = np.asarray(inputs["Wk"], np.float32)
    Wv = np.asarray(inputs["Wv"], np.float32)
    Wo = np.asarray(inputs["Wo"], np.float32)

    per_batch = []
    for b in range(B):
        per_batch.append({
            "qT": np.ascontiguousarray(q[b].T).astype(nbf16),
            "kT": np.ascontiguousarray(k[b].T).astype(nbf16),
            "vT": np.ascontiguousarray(v[b].T).astype(nbf16),
            "maskT": np.ascontiguousarray(
                (~mask[b].astype(bool)).T).astype(nbf16),
        })
    per_group = []
    for g in range(4):
        sl = slice(g * GD, (g + 1) * GD)
        per_group.append({
            "wqT": np.ascontiguousarray(Wq[sl, :].T).astype(nbf16),
            "wkT": np.ascontiguousarray(Wk[sl, :].T).astype(nbf16),
            "wvT": np.ascontiguousarray(Wv[sl, :].T).astype(nbf16),
            "woT": np.ascontiguousarray(Wo[:, sl].T).astype(nbf16),
        })
    in_maps = []
    for c in range(N_CORES):
        b, g = c // 4, c % 4
        m = {}
        m.update(per_batch[b])
        m.update(per_group[g])
        in_maps.append(m)
    return in_maps


def assemble(results):
    out = np.zeros((B, S, D), np.float32)
    kp = np.empty((B, S, D), np.float32)
    vp = np.empty((B, S, D), np.float32)
    for c, res in enumerate(results):
        b, g = c // 4, c % 4
        kp[b][:, g * GD:(g + 1) * GD] = res["kpT_out"].T
        vp[b][:, g * GD:(g + 1) * GD] = res["vp_out"]
        if USE_RS:
            for t in range(NB):
                out[b, t * BLK + g * 128: t * BLK + (g + 1) * 128, :] = \
                    res["out_rs"][t * 128:(t + 1) * 128, :]
        else:
            out[b] += res["outp_out"].astype(np.float32)
    return out, kp, vp


def run_cores(in_maps, trace=False, **kwargs):
    nc = get_nc()
    return bass_utils.run_bass_kernel_spmd(
        nc, in_maps, core_ids=list(range(N_CORES)), trace=trace, **kwargs
    )


def kernel(**inputs):
    in_maps = make_in_maps(inputs)
    res = run_cores(in_maps, trace=False)
    return assemble(res.results)



# revision 4
# speedup vs baseline: 1.1176x; 1.1176x over previous
"""Distributed Trainium2 kernel for nn_AttentionLayer (B=2, S=2048, D=2048, H=16).

Sharding: core c = (batch b, head-group g) with b = c // 4, g = c % 4.
Each core owns 4 heads (512 of the 2048 projection dims) of one batch element:
  - computes qp/kp/vp for its head-group (bf16 matmuls, f32 accumulation),
  - masked softmax attention for its 4 heads (no max-subtraction: scores are
    bounded; masked entries become exactly 0 via exp(s) * (1-mask)),
  - its partial output projection (Wo row-shard).
The 4 partial outputs per batch are summed on the host (the axon PJRT path in
this container cannot execute cross-core collectives). Host side otherwise
only transposes/casts/concatenates.

Inputs staged per core (host pre-transposed, bf16):
  qT/kT/vT [D, S], maskT [S, S] (=1-mask.T), wqT/wkT/wvT [D, 512], woT [512, D]
Outputs per core: kpT_out [512, S] f32, vp_out [S, 512] f32,
  outp_out [S, D] bf16 partial (host upcasts + sums).

Schedule (v2): all pools coexist (no phase barrier).  Scores PSUM is
allocated as [128,1024] two-bank groups packing two key-chunks, so each
exp is one fat ACT op and each mask-mul one fat DVE op.  The first two
score steps are interleaved into the v-projection so the scalar engine
starts the softmax chain ~35us before the attention phase begins, and
the attention loop runs a lookahead-2 software pipeline:
scores(i+2) -> ctx(i) -> (outproj at end of each t-block).
Engine split: scalar = bf16 psum->sbuf copies + exp; vector = f32 output
staging, mask-mul, normalize, transpose drain; tensor = matmuls + ctx
transposes.
"""

import numpy as np
import ml_dtypes

import concourse.bass as bass  # noqa: F401
import concourse.mybir as mybir
import concourse.tile as tile
from concourse import bacc
from concourse import bass_utils
from concourse.masks import make_identity

BF16 = mybir.dt.bfloat16
F32 = mybir.dt.float32
nbf16 = ml_dtypes.bfloat16

B, S, D, H = 2, 2048, 2048, 16
GH = 4                # heads per core
DH = 128              # head dim
GD = GH * DH          # 512 local projection dims
KC = D // 128         # 16 contraction chunks
NB = 4                # query blocks
BLK = S // NB         # 512
NG = KC // 2          # 8 score groups per step (2 key-chunks each)
N_CORES = 8
SCALE = float(1.0 / np.sqrt(DH))

_CACHE = {}


def _build():
    nc = bacc.Bacc(
        "TRN2", target_bir_lowering=False, debug=False, num_devices=N_CORES
    )
    AF = mybir.ActivationFunctionType

    qT = nc.dram_tensor("qT", [D, S], BF16, kind="ExternalInput")
    kT = nc.dram_tensor("kT", [D, S], BF16, kind="ExternalInput")
    vT = nc.dram_tensor("vT", [D, S], BF16, kind="ExternalInput")
    maskT = nc.dram_tensor("maskT", [S, S], BF16, kind="ExternalInput")
    wqT = nc.dram_tensor("wqT", [D, GD], BF16, kind="ExternalInput")
    wkT = nc.dram_tensor("wkT", [D, GD], BF16, kind="ExternalInput")
    wvT = nc.dram_tensor("wvT", [D, GD], BF16, kind="ExternalInput")
    woT = nc.dram_tensor("woT", [GD, D], BF16, kind="ExternalInput")
    kpT_out = nc.dram_tensor("kpT_out", [GD, S], F32, kind="ExternalOutput")
    vp_out = nc.dram_tensor("vp_out", [S, GD], F32, kind="ExternalOutput")
    outp_out = nc.dram_tensor("outp_out", [S, D], BF16, kind="ExternalOutput")

    with tile.TileContext(nc) as tc:
        with (
            tc.tile_pool(name="res", bufs=1) as res,
            tc.tile_pool(name="mp", bufs=16) as mp,
            tc.tile_pool(name="apl", bufs=18) as apl,
            tc.tile_pool(name="cpl", bufs=8) as cpl,
            tc.tile_pool(name="stream", bufs=6) as stream,
            tc.tile_pool(name="stageA", bufs=6) as stageA,
            tc.tile_pool(name="stageB", bufs=6) as stageB,
            tc.tile_pool(name="psS", bufs=2, space="PSUM") as psS,
        ):
            # ---- resident SBUF tensors ----
            wo_sb = res.tile([128, GH * D], BF16, name="wo_sb", tag="wo")
            qp_sb = res.tile([128, GH * S], BF16, name="qp_sb", tag="qp")
            kp_sb = res.tile([128, GH * S], BF16, name="kp_sb", tag="kp")
            # vp + per-head ones column: s-chunk sc at cols sc*516, head h at
            # +h*129 (128 vp dims then one 1.0 column for the softmax denom)
            vpo_sb = res.tile([128, KC * 516], BF16, name="vpo_sb", tag="vpo")
            ident = res.tile([128, 128], BF16, name="ident", tag="ident")

            make_identity(nc, ident[:])
            nc.gpsimd.memset(vpo_sb[:], 1.0)

            mask_tiles = {}   # t -> list of NG [128,1024] tiles
            attn_tiles = {}   # (t, h) -> list of NG [128,1024] tiles
            ctx_tiles = {}    # t -> {h: cth}

            def emit_mask_dmas(t):
                mts = []
                for g in range(NG):
                    mt = mp.tile([128, 1024], BF16, name="mt", tag="mt")
                    for j in range(2):
                        skc = 2 * g + j
                        nc.sync.dma_start(
                            out=mt[:, j * 512:(j + 1) * 512],
                            in_=maskT[skc * 128:(skc + 1) * 128,
                                      t * 512:(t + 1) * 512],
                        )
                    mts.append(mt)
                mask_tiles[t] = mts

            def emit_scores(t, h):
                mts = mask_tiles[t]
                attn = []
                for g in range(NG):
                    sps = psS.tile([128, 1024], F32, name="sps", tag="sps")
                    for j in range(2):
                        skc = 2 * g + j
                        nc.tensor.matmul(
                            sps[:, j * 512:(j + 1) * 512],
                            lhsT=kp_sb[:, h * S + skc * 128:
                                       h * S + (skc + 1) * 128],
                            rhs=qp_sb[:, h * S + t * 512: h * S + (t + 1) * 512],
                            start=True,
                            stop=True,
                        )
                    at = apl.tile([128, 1024], BF16, name="at", tag="at")
                    nc.scalar.activation(at[:], sps[:], AF.Exp, scale=SCALE)
                    nc.vector.tensor_mul(at[:], at[:], mts[g][:])
                    attn.append(at)
                attn_tiles[(t, h)] = attn

            def emit_ctx(t, h):
                attn = attn_tiles.pop((t, h))
                ct = ctx_tiles.setdefault(t, {})
                cth = cpl.tile([128, 512], BF16, name="ct", tag="ct")
                for mm in range(4):
                    cps = psCT.tile([128, 129], F32, name="cps", tag="psct")
                    for skc in range(KC):
                        g, j = skc // 2, skc % 2
                        nc.tensor.matmul(
                            cps[:],
                            lhsT=attn[g][:, j * 512 + mm * 128:
                                         j * 512 + (mm + 1) * 128],
                            rhs=vpo_sb[:, skc * 516 + h * 129:
                                       skc * 516 + (h + 1) * 129],
                            start=(skc == 0),
                            stop=(skc == KC - 1),
                        )
                    rec = stageB.tile([128, 1], F32, name="rec", tag="rec")
                    nc.vector.reciprocal(rec[:], cps[:, 128:129])
                    ctn = stageB.tile([128, 128], BF16, name="ctn", tag="ctn")
                    nc.vector.tensor_scalar_mul(ctn[:], cps[:, 0:128], rec[:])
                    tps = psCT.tile([128, 128], BF16, name="tps", tag="psct")
                    nc.tensor.transpose(tps[:], ctn[:], ident[:])
                    nc.vector.tensor_copy(cth[:, mm * 128:(mm + 1) * 128],
                                          tps[:])
                ct[h] = cth

            def emit_outproj(t):
                cts = ctx_tiles.pop(t)
                # LDW amortization: each ct stationary serves an n-pair
                for mm in range(4):
                    for npair in range(2):
                        ops2 = [psO.tile([128, 512], F32, name=f"ops{j}",
                                         tag="ops") for j in range(2)]
                        for h in range(GH):
                            for n2 in range(2):
                                n = npair * 2 + n2
                                nc.tensor.matmul(
                                    ops2[n2][:],
                                    lhsT=cts[h][:, mm * 128:(mm + 1) * 128],
                                    rhs=wo_sb[:, h * D + n * 512:
                                              h * D + (n + 1) * 512],
                                    start=(h == 0),
                                    stop=(h == GH - 1),
                                )
                        for n2 in range(2):
                            n = npair * 2 + n2
                            ost = stageB.tile([128, 512], BF16,
                                              name="ost", tag="ost")
                            # alternate engines to halve the drain tail
                            if n2 == 0:
                                nc.vector.tensor_copy(ost[:], ops2[n2][:])
                            else:
                                nc.scalar.copy(ost[:], ops2[n2][:])
                            nc.sync.dma_start(
                                out=outp_out[t * BLK + mm * 128:
                                             t * BLK + (mm + 1) * 128,
                                             n * 512:(n + 1) * 512],
                                in_=ost[:],
                            )

            # ---------------- Phase A: projections ----------------
            with (
                tc.tile_pool(name="wpool", bufs=2) as wpool,
                tc.tile_pool(name="psA", bufs=4, space="PSUM") as psA,
            ):
                wq_sb = wpool.tile([128, KC * GD], BF16, name="wq_sb", tag="w")
                wk_sb = wpool.tile([128, KC * GD], BF16, name="wk_sb", tag="w")

                # qpT / kpT in [dout, s] orientation.  Per 512-seq block n:
                # 4 m-psums in flight; stationary (kc, m) serves one N=512
                # matmul (FWL hides the per-matmul LDWEIGHTS).
                for xdram, wdram, w_sb, dst_bf, f32out in (
                    (qT, wqT, wq_sb, qp_sb, None),
                    (kT, wkT, wk_sb, kp_sb, kpT_out),
                ):
                    for n in range(4):
                        pss = [
                            psA.tile([128, 512], F32, name=f"psa{m}", tag="psa")
                            for m in range(4)
                        ]
                        for kc in range(KC):
                            if n == 0:
                                nc.sync.dma_start(
                                    out=w_sb[:, kc * GD:(kc + 1) * GD],
                                    in_=wdram[kc * 128:(kc + 1) * 128, :],
                                )
                            xs = stream.tile([128, 512], BF16, name="xs",
                                             tag="xs")
                            nc.sync.dma_start(
                                out=xs[:],
                                in_=xdram[kc * 128:(kc + 1) * 128,
                                          n * 512:(n + 1) * 512],
                            )
                            for m in range(4):
                                nc.tensor.matmul(
                                    pss[m][:],
                                    lhsT=w_sb[:, kc * GD + m * 128:
                                              kc * GD + (m + 1) * 128],
                                    rhs=xs[:],
                                    start=(kc == 0),
                                    stop=(kc == KC - 1),
                                )
                        for m in range(4):
                            nc.scalar.copy(
                                dst_bf[:, m * S + n * 512:
                                       m * S + (n + 1) * 512],
                                pss[m][:],
                            )
                            if f32out is not None:
                                st = stageA.tile([128, 512], F32, name="st",
                                                 tag="st")
                                nc.vector.tensor_copy(st[:], pss[m][:])
                                nc.sync.dma_start(
                                    out=f32out[m * 128:(m + 1) * 128,
                                               n * 512:(n + 1) * 512],
                                    in_=st[:],
                                )
                        if xdram is qT and n == 0:
                            # early mask prefetch for t=0 and t=1 on spare
                            # DMA bandwidth while the projections compute
                            emit_mask_dmas(0)
                            emit_mask_dmas(1)

                # weights for v / out-proj
                wv_sb = wpool.tile([128, KC * GD], BF16, name="wv_sb", tag="w")
                for kc in range(KC):
                    nc.sync.dma_start(
                        out=wv_sb[:, kc * GD:(kc + 1) * GD],
                        in_=wvT[kc * 128:(kc + 1) * 128, :],
                    )
                for h in range(GH):
                    nc.sync.dma_start(
                        out=wo_sb[:, h * D:(h + 1) * D],
                        in_=woT[h * 128:(h + 1) * 128, :],
                    )
                # vp in [s, dout] orientation, interleaved with the first two
                # score steps so ACT/DVE start the softmax chain early.
                for mq in range(4):
                    pss = [
                        psA.tile([128, 512], F32, name=f"psa{m}", tag="psa")
                        for m in range(4)
                    ]
                    for kc in range(KC):
                        xs = stream.tile([128, 512], BF16, name="xs", tag="xs")
                        nc.sync.dma_start(
                            out=xs[:],
                            in_=vT[kc * 128:(kc + 1) * 128,
                                   mq * 512:(mq + 1) * 512],
                        )
                        for mi in range(4):
                            nc.tensor.matmul(
                                pss[mi][:],
                                lhsT=xs[:, mi * 128:(mi + 1) * 128],
                                rhs=wv_sb[:, kc * GD:(kc + 1) * GD],
                                start=(kc == 0),
                                stop=(kc == KC - 1),
                            )
                    for mi in range(4):
                        sc = mq * 4 + mi
                        for h in range(GH):
                            nc.scalar.copy(
                                vpo_sb[:, sc * 516 + h * 129:
                                       sc * 516 + h * 129 + 128],
                                pss[mi][:, h * 128:(h + 1) * 128],
                            )
                        st = stageA.tile([128, 512], F32, name="st", tag="st")
                        nc.vector.tensor_copy(st[:], pss[mi][:])
                        nc.sync.dma_start(
                            out=vp_out[sc * 128:(sc + 1) * 128, :], in_=st[:]
                        )
                    if mq == 1:
                        emit_scores(0, 0)
                    elif mq == 3:
                        emit_scores(0, 1)

            # ---------------- Phase B: attention + out-proj ----------------
            with (
                tc.tile_pool(name="psCT", bufs=2, space="PSUM") as psCT,
                tc.tile_pool(name="psO", bufs=2, space="PSUM") as psO,
            ):
                # lookahead-2 pipeline: scores(i+2) before ctx(i)
                steps = [(t, h) for t in range(NB) for h in range(GH)]
                emit_scores(0, 2)
                for i in range(len(steps)):
                    la = i + 3  # scores for steps 0..2 already emitted
                    if la < len(steps):
                        lt, lh = steps[la]
                        if lh == 0 and lt >= 2:
                            emit_mask_dmas(lt)
                        emit_scores(lt, lh)
                    t, h = steps[i]
                    emit_ctx(t, h)
                    if h == GH - 1:
                        emit_outproj(t)

    nc.compile()
    return nc


def get_nc():
    if "nc" not in _CACHE:
        _CACHE["nc"] = _build()
    return _CACHE["nc"]


def make_in_maps(inputs):
    q = np.asarray(inputs["q"], np.float32)
    k = np.asarray(inputs["k"], np.float32)
    v = np.asarray(inputs["v"], np.float32)
    mask = np.asarray(inputs["mask"])
    Wq = np.asarray(inputs["Wq"], np.float32)
    Wk = np.asarray(inputs["Wk"], np.float32)
    Wv = np.asarray(inputs["Wv"], np.float32)
    Wo = np.asarray(inputs["Wo"], np.float32)

    per_batch = []
    for b in range(B):
        per_batch.append({
            "qT": np.ascontiguousarray(q[b].T).astype(nbf16),
            "kT": np.ascontiguousarray(k[b].T).astype(nbf16),
            "vT": np.ascontiguousarray(v[b].T).astype(nbf16),
            "maskT": np.ascontiguousarray(
                (~mask[b].astype(bool)).T).astype(nbf16),
        })
    per_group = []
    for g in range(4):
        sl = slice(g * GD, (g + 1) * GD)
        per_group.append({
            "wqT": np.ascontiguousarray(Wq[sl, :].T).astype(nbf16),
            "wkT": np.ascontiguousarray(Wk[sl, :].T).astype(nbf16),
            "wvT": np.ascontiguousarray(Wv[sl, :].T).astype(nbf16),
            "woT": np.ascontiguousarray(Wo[:, sl].T).astype(nbf16),
        })
    in_maps = []
    for c in range(N_CORES):
        b, g = c // 4, c % 4
        m = {}
        m.update(per_batch[b])
        m.update(per_group[g])
        in_maps.append(m)
    return in_maps


def assemble(results):
    out = np.zeros((B, S, D), np.float32)
    kp = np.empty((B, S, D), np.float32)
    vp = np.empty((B, S, D), np.float32)
    for c, res in enumerate(results):
        b, g = c // 4, c % 4
        kp[b][:, g * GD:(g + 1) * GD] = res["kpT_out"].T
        vp[b][:, g * GD:(g + 1) * GD] = res["vp_out"]
        out[b] += res["outp_out"].astype(np.float32)
    return out, kp, vp


def run_cores(in_maps, trace=False, **kwargs):
    nc = get_nc()
    return bass_utils.run_bass_kernel_spmd(
        nc, in_maps, core_ids=list(range(N_CORES)), trace=trace, **kwargs
    )


def kernel(**inputs):
    in_maps = make_in_maps(inputs)
    res = run_cores(in_maps, trace=False)
    return assemble(res.results)


# revision 50
# speedup vs baseline: 1.2919x; 1.1559x over previous
"""Distributed Trainium2 kernel for nn_AttentionLayer (B=2, S=2048, D=2048, H=16).

Sharding: core c = (batch b, head-group g) with b = c // 4, g = c % 4.
Each core owns 4 heads (512 of the 2048 projection dims) of one batch element:
  - computes qp/kp/vp for its head-group (bf16 matmuls, f32 accumulation),
  - masked softmax attention for its 4 heads (no max-subtraction: scores are
    bounded; masked entries become exactly 0 via exp(s) * (1-mask)),
  - its partial output projection (Wo row-shard).
The 4 partial outputs per batch are summed on the host (the axon PJRT path in
this container cannot execute cross-core collectives). Host side otherwise
only transposes/casts/concatenates.

Inputs staged per core (host pre-transposed, bf16):
  qT/kT/vT [D, S], maskT [S, S] (=1-mask.T), wqT/wkT/wvT [D, 512], woT [512, D]
Outputs per core: kpT_out [512, S] f32, vp_out [S, 512] f32,
  outp_out [S, D] bf16 partial (host upcasts + sums).

Schedule (v2): all pools coexist (no phase barrier).  Scores PSUM is
allocated as [128,1024] two-bank groups packing two key-chunks, so each
exp is one fat ACT op and each mask-mul one fat DVE op.  The first two
score steps are interleaved into the v-projection so the scalar engine
starts the softmax chain ~35us before the attention phase begins, and
the attention loop runs a lookahead-2 software pipeline:
scores(i+2) -> ctx(i) -> (outproj at end of each t-block).
Engine split: scalar = bf16 psum->sbuf copies + exp; vector = f32 output
staging, mask-mul, normalize, transpose drain; tensor = matmuls + ctx
transposes.
"""

import numpy as np
import ml_dtypes

import concourse.bass as bass  # noqa: F401
import concourse.mybir as mybir
import concourse.tile as tile
from concourse import bacc
from concourse import bass_utils
from concourse.masks import make_identity

BF16 = mybir.dt.bfloat16
F32 = mybir.dt.float32
nbf16 = ml_dtypes.bfloat16

B, S, D, H = 2, 2048, 2048, 16
GH = 4                # heads per core
DH = 128              # head dim
GD = GH * DH          # 512 local projection dims
KC = D // 128         # 16 contraction chunks
NB = 4                # query blocks
BLK = S // NB         # 512
NG = KC // 2          # 8 score groups per step (2 key-chunks each)
N_CORES = 8
SCALE = float(1.0 / np.sqrt(DH))

_CACHE = {}


def _build():
    nc = bacc.Bacc(
        "TRN2", target_bir_lowering=False, debug=False, num_devices=N_CORES
    )
    AF = mybir.ActivationFunctionType

    qT = nc.dram_tensor("qT", [D, S], BF16, kind="ExternalInput")
    kT = nc.dram_tensor("kT", [D, S], BF16, kind="ExternalInput")
    vT = nc.dram_tensor("vT", [D, S], BF16, kind="ExternalInput")
    maskT = nc.dram_tensor("maskT", [S, S], BF16, kind="ExternalInput")
    wqT = nc.dram_tensor("wqT", [D, GD], BF16, kind="ExternalInput")
    wkT = nc.dram_tensor("wkT", [D, GD], BF16, kind="ExternalInput")
    wvT = nc.dram_tensor("wvT", [D, GD], BF16, kind="ExternalInput")
    woT = nc.dram_tensor("woT", [GD, D], BF16, kind="ExternalInput")
    kpT_out = nc.dram_tensor("kpT_out", [GD, S], BF16, kind="ExternalOutput")
    vp_out = nc.dram_tensor("vp_out", [S, GD], BF16, kind="ExternalOutput")
    outp_out = nc.dram_tensor("outp_out", [S, D], BF16, kind="ExternalOutput")

    with tile.TileContext(nc) as tc:
        with (
            tc.tile_pool(name="res", bufs=1) as res,
            tc.tile_pool(name="mp", bufs=12) as mp,
            tc.tile_pool(name="apl", bufs=14) as apl,
            tc.tile_pool(name="cpl", bufs=8) as cpl,
            tc.tile_pool(name="stream", bufs=7) as stream,
            tc.tile_pool(name="stageA", bufs=5) as stageA,
            tc.tile_pool(name="stageB", bufs=8) as stageB,
            tc.tile_pool(name="psS", bufs=2, space="PSUM") as psS,
        ):
            # ---- resident SBUF tensors ----
            wo_sb = res.tile([128, GH * D], BF16, name="wo_sb", tag="wo")
            qp_sb = res.tile([128, GH * S], BF16, name="qp_sb", tag="qp")
            kp_sb = res.tile([128, GH * S], BF16, name="kp_sb", tag="kp")
            # vp + per-head ones column: s-chunk sc at cols sc*516, head h at
            # +h*129 (128 vp dims then one 1.0 column for the softmax denom)
            vpo_sb = res.tile([128, KC * 516], BF16, name="vpo_sb", tag="vpo")
            ident = res.tile([128, 128], BF16, name="ident", tag="ident")

            # HAM warm-up: the first ~6us are DMA-bound; keep the PE busy on
            # scratch data (DVE memset starts immediately — gpsimd takes
            # ~14us to come up) so the clock gate is at full rate when real
            # work arrives.  Ping-pong psum banks so the matmuls pipeline.
            scr = res.tile([128, 128], BF16, name="scr", tag="scr")
            nc.vector.memset(scr[:], 0.125)
            warm = psS.tile([128, 1024], F32, name="warm", tag="sps")
            for i in range(24):
                half = (i % 2) * 512
                nc.tensor.matmul(warm[:, half:half + 128], lhsT=scr[:],
                                 rhs=scr[:], start=True, stop=True)

            make_identity(nc, ident[:])
            nc.gpsimd.memset(vpo_sb[:], 1.0)

            mask_tiles = {}   # t -> list of NG [128,1024] tiles
            attn_tiles = {}   # (t, h) -> list of NG [128,1024] tiles
            ctx_tiles = {}    # t -> {h: cth}

            def emit_mask_group(t, g):
                # one [128,1024] mask tile (two key-chunks), one descriptor —
                # DMA descriptors are expensive, so pack both chunks into a
                # single 3D-AP transfer
                mt = mp.tile([128, 1024], BF16, name="mt", tag="mt")
                nc.sync.dma_start(
                    out=mt.rearrange("p (j c) -> p j c", j=2),
                    in_=maskT[2 * g * 128:(2 * g + 2) * 128,
                              t * 512:(t + 1) * 512]
                    .rearrange("(j p) c -> p j c", j=2),
                )
                mask_tiles.setdefault(t, [None] * NG)[g] = mt

            def emit_scores(t, h):
                mts = mask_tiles[t]
                attn = []
                for g in range(NG):
                    sps = psS.tile([128, 1024], F32, name="sps", tag="sps")
                    for j in range(2):
                        skc = 2 * g + j
                        nc.tensor.matmul(
                            sps[:, j * 512:(j + 1) * 512],
                            lhsT=kp_sb[:, h * S + skc * 128:
                                       h * S + (skc + 1) * 128],
                            rhs=qp_sb[:, h * S + t * 512: h * S + (t + 1) * 512],
                            start=True,
                            stop=True,
                        )
                    at = apl.tile([128, 1024], BF16, name="at", tag="at")
                    nc.scalar.activation(at[:], sps[:], AF.Exp, scale=SCALE)
                    nc.vector.tensor_mul(at[:], at[:], mts[g][:])
                    attn.append(at)
                attn_tiles[(t, h)] = attn

            def emit_ctx(t, h):
                attn = attn_tiles.pop((t, h))
                ct = ctx_tiles.setdefault(t, {})
                cth = cpl.tile([128, 512], BF16, name="ct", tag="ct")
                for mm in range(4):
                    cps = psCT.tile([128, 129], F32, name="cps", tag="psct")
                    for skc in range(KC):
                        g, j = skc // 2, skc % 2
                        nc.tensor.matmul(
                            cps[:],
                            lhsT=attn[g][:, j * 512 + mm * 128:
                                         j * 512 + (mm + 1) * 128],
                            rhs=vpo_sb[:, skc * 516 + h * 129:
                                       skc * 516 + (h + 1) * 129],
                            start=(skc == 0),
                            stop=(skc == KC - 1),
                        )
                    rec = stageB.tile([128, 1], F32, name="rec", tag="rec")
                    nc.vector.reciprocal(rec[:], cps[:, 128:129])
                    ctn = stageB.tile([128, 128], BF16, name="ctn", tag="ctn")
                    nc.vector.tensor_scalar_mul(ctn[:], cps[:, 0:128], rec[:])
                    tps = psCT.tile([128, 128], BF16, name="tps", tag="psct")
                    nc.tensor.transpose(tps[:], ctn[:], ident[:])
                    nc.vector.tensor_copy(cth[:, mm * 128:(mm + 1) * 128],
                                          tps[:])
                ct[h] = cth

            def emit_outproj_chunk(t, c):
                # chunk c = (mm, npair): 8 matmuls + one paired output DMA.
                # Chunks are spread across the next t-block's steps so the
                # out-projection fills PE gaps in the ctx normalize chains.
                mm, npair = c // 2, c % 2
                cts = ctx_tiles[t]
                ops2 = [psO.tile([128, 512], F32, name=f"ops{j}",
                                 tag="ops") for j in range(2)]
                for h in range(GH):
                    for n2 in range(2):
                        n = npair * 2 + n2
                        nc.tensor.matmul(
                            ops2[n2][:],
                            lhsT=cts[h][:, mm * 128:(mm + 1) * 128],
                            rhs=wo_sb[:, h * D + n * 512:
                                      h * D + (n + 1) * 512],
                            start=(h == 0),
                            stop=(h == GH - 1),
                        )
                ost = stageB.tile([128, 1024], BF16, name="ost", tag="ost")
                # alternate engines to halve the drain tail
                nc.vector.tensor_copy(ost[:, 0:512], ops2[0][:])
                nc.scalar.copy(ost[:, 512:1024], ops2[1][:])
                if t == NB - 1 and c >= 6:
                    # only the last stores are latency-exposed; split them
                    # across two queues (descriptor issue itself costs
                    # ~640ns, so don't split more)
                    for w in range(2):
                        nc.sync.dma_start(
                            out=outp_out[
                                t * BLK + mm * 128 + w * 64:
                                t * BLK + mm * 128 + (w + 1) * 64,
                                npair * 1024:(npair + 1) * 1024],
                            in_=ost[w * 64:(w + 1) * 64, :],
                        )
                else:
                    nc.sync.dma_start(
                        out=outp_out[t * BLK + mm * 128:
                                     t * BLK + (mm + 1) * 128,
                                     npair * 1024:(npair + 1) * 1024],
                        in_=ost[:],
                    )
                if c == 7:
                    ctx_tiles.pop(t)

            # ---------------- Phase A: projections ----------------
            with (
                tc.tile_pool(name="wpool", bufs=1) as wpool,
                tc.tile_pool(name="psA", bufs=4, space="PSUM") as psA,
            ):
                # wq/wv share a slot ring (wv loads after q-proj drains wq);
                # wk gets its own so it can prefetch during q-proj.
                wq_sb = wpool.tile([128, KC * GD], BF16, name="wq_sb",
                                   tag="wqv")
                wk_sb = wpool.tile([128, KC * GD], BF16, name="wk_sb",
                                   tag="wk")
                # wv reuses wq's slot; its prefetch DMA (during k-proj) only
                # needs q-proj's weight reads to have completed
                wv_sb = wpool.tile([128, KC * GD], BF16, name="wv_sb",
                                   tag="wqv")

                def w_chunk_dma(w_sb, wdram, kc, ways=1):
                    # ways>1 only for the latency-critical first chunks
                    step = 128 // ways
                    for w in range(ways):
                        nc.sync.dma_start(
                            out=w_sb[w * step:(w + 1) * step,
                                     kc * GD:(kc + 1) * GD],
                            in_=wdram[kc * 128 + w * step:
                                      kc * 128 + (w + 1) * step, :],
                        )

                def w_pair_dma(w_sb, wdram, kcp):
                    # two weight chunks in one descriptor
                    nc.sync.dma_start(
                        out=w_sb[:, 2 * kcp * GD:(2 * kcp + 2) * GD]
                        .rearrange("p (j c) -> p j c", j=2),
                        in_=wdram[2 * kcp * 128:(2 * kcp + 2) * 128, :]
                        .rearrange("(j p) c -> p j c", j=2),
                    )

                # qpT / kpT in [dout, s] orientation.  Per 512-seq block n:
                # 4 m-psums in flight; stationary (kc, m) serves one N=512
                # matmul (FWL hides the per-matmul LDWEIGHTS).
                for xdram, wdram, w_sb, dst_bf, bfout in (
                    (qT, wqT, wq_sb, qp_sb, None),
                    (kT, wkT, wk_sb, kp_sb, kpT_out),
                ):
                    for n in range(4):
                        pss = [
                            psA.tile([128, 512], F32, name=f"psa{m}",
                                     tag="psa")
                            for m in range(4)
                        ]
                        for kcp in range(KC // 2):
                            xs = stream.tile([128, 1024], BF16, name="xs",
                                             tag="xs")
                            if xdram is qT and n == 0 and kcp < 2:
                                # first activation triggers go out before
                                # the weight triggers (sync issues one
                                # descriptor per ~640ns), unpaired so the
                                # first tiles arrive on 2 queues each
                                for j in range(2):
                                    nc.sync.dma_start(
                                        out=xs[:, j * 512:(j + 1) * 512],
                                        in_=xdram[(2 * kcp + j) * 128:
                                                  (2 * kcp + j + 1) * 128,
                                                  0:512],
                                    )
                                if kcp == 0:
                                    w_chunk_dma(w_sb, wdram, 0, ways=2)
                                    w_chunk_dma(w_sb, wdram, 1, ways=2)
                                else:
                                    w_pair_dma(w_sb, wdram, kcp)
                            else:
                                if n == 0 and xdram is qT:
                                    w_pair_dma(w_sb, wdram, kcp)
                                nc.sync.dma_start(
                                    out=xs.rearrange("p (j c) -> p j c", j=2),
                                    in_=xdram[2 * kcp * 128:
                                              (2 * kcp + 2) * 128,
                                              n * 512:(n + 1) * 512]
                                    .rearrange("(j p) c -> p j c", j=2),
                                )
                            for j in range(2):
                                kc = 2 * kcp + j
                                for m in range(4):
                                    nc.tensor.matmul(
                                        pss[m][:],
                                        lhsT=w_sb[:, kc * GD + m * 128:
                                                  kc * GD + (m + 1) * 128],
                                        rhs=xs[:, j * 512:(j + 1) * 512],
                                        start=(kc == 0),
                                        stop=(kc == KC - 1),
                                    )
                            # prefetch next-projection weights with a full
                            # n-block of lead time
                            if xdram is qT and n == 2:
                                w_pair_dma(wk_sb, wkT, kcp)
                            elif xdram is kT and n == 2:
                                w_pair_dma(wv_sb, wvT, kcp)
                        for mp2 in range(2):
                            st = (stageA.tile([128, 1024], BF16, name="st",
                                              tag="st")
                                  if bfout is not None else None)
                            for m2 in range(2):
                                m = 2 * mp2 + m2
                                nc.scalar.copy(
                                    dst_bf[:, m * S + n * 512:
                                           m * S + (n + 1) * 512],
                                    pss[m][:],
                                )
                                if st is not None:
                                    nc.vector.tensor_copy(
                                        st[:, m2 * 512:(m2 + 1) * 512],
                                        pss[m][:],
                                    )
                            if st is not None:
                                nc.sync.dma_start(
                                    out=bfout[2 * mp2 * 128:
                                              (2 * mp2 + 2) * 128,
                                              n * 512:(n + 1) * 512]
                                    .rearrange("(j p) c -> p j c", j=2),
                                    in_=st.rearrange("p (j c) -> p j c", j=2),
                                )
                        if xdram is kT:
                            # t=0 mask prefetch, two groups per seq-block,
                            # trickled in on spare DMA bandwidth
                            emit_mask_group(0, 2 * n)
                            emit_mask_group(0, 2 * n + 1)

                for h in range(GH):
                    nc.sync.dma_start(
                        out=wo_sb[:, h * D:(h + 1) * D],
                        in_=woT[h * 128:(h + 1) * 128, :],
                    )
                # vp in [s, dout] orientation, interleaved with the first two
                # score steps so ACT/DVE start the softmax chain early.
                for mq in range(4):
                    pss = [
                        psA.tile([128, 512], F32, name=f"psa{m}", tag="psa")
                        for m in range(4)
                    ]
                    for kcp in range(KC // 2):
                        xs = stream.tile([128, 1024], BF16, name="xs",
                                         tag="xs")
                        nc.sync.dma_start(
                            out=xs.rearrange("p (j c) -> p j c", j=2),
                            in_=vT[2 * kcp * 128:(2 * kcp + 2) * 128,
                                   mq * 512:(mq + 1) * 512]
                            .rearrange("(j p) c -> p j c", j=2),
                        )
                        for j in range(2):
                            kc = 2 * kcp + j
                            for mi in range(4):
                                nc.tensor.matmul(
                                    pss[mi][:],
                                    lhsT=xs[:, j * 512 + mi * 128:
                                            j * 512 + (mi + 1) * 128],
                                    rhs=wv_sb[:, kc * GD:(kc + 1) * GD],
                                    start=(kc == 0),
                                    stop=(kc == KC - 1),
                                )
                    # scores before the drain: their exp must not queue
                    # behind the vpo copies on the scalar engine
                    if mq == 1:
                        emit_scores(0, 0)
                    elif mq == 3:
                        emit_scores(0, 1)
                    for mp2 in range(2):
                        st = stageA.tile([128, 1024], BF16, name="st",
                                         tag="st")
                        for m2 in range(2):
                            mi = 2 * mp2 + m2
                            sc = mq * 4 + mi
                            for h in range(GH):
                                # late blocks drain on vector only: the
                                # scalar engine must stay free for exp on
                                # the A->B critical path
                                eng_copy = (nc.scalar.copy
                                            if (h % 2 and mq < 2) else
                                            nc.vector.tensor_copy)
                                eng_copy(
                                    vpo_sb[:, sc * 516 + h * 129:
                                           sc * 516 + h * 129 + 128],
                                    pss[mi][:, h * 128:(h + 1) * 128],
                                )
                            nc.vector.tensor_copy(
                                st[:, m2 * 512:(m2 + 1) * 512], pss[mi][:])
                        sc0 = mq * 4 + 2 * mp2
                        nc.sync.dma_start(
                            out=vp_out[sc0 * 128:(sc0 + 2) * 128, :]
                            .rearrange("(j p) c -> p j c", j=2),
                            in_=st.rearrange("p (j c) -> p j c", j=2),
                        )
                    emit_mask_group(1, 2 * mq)
                    emit_mask_group(1, 2 * mq + 1)

            # ---------------- Phase B: attention + out-proj ----------------
            with (
                tc.tile_pool(name="psCT", bufs=2, space="PSUM") as psCT,
                tc.tile_pool(name="psO", bufs=2, space="PSUM") as psO,
            ):
                # lookahead-2 pipeline: scores(i+2) before ctx(i); outproj of
                # the previous block spread two chunks per step
                steps = [(t, h) for t in range(NB) for h in range(GH)]
                emit_scores(0, 2)
                for i in range(len(steps)):
                    t, h = steps[i]
                    # prefetch mask(t+2) two groups per step of block t
                    if t + 2 < NB:
                        emit_mask_group(t + 2, 2 * h)
                        emit_mask_group(t + 2, 2 * h + 1)
                    la = i + 3  # scores for steps 0..2 already emitted
                    if la < len(steps):
                        emit_scores(*steps[la])
                    if t > 0:
                        emit_outproj_chunk(t - 1, 2 * h)
                        emit_outproj_chunk(t - 1, 2 * h + 1)
                    emit_ctx(t, h)
                for c in range(8):
                    emit_outproj_chunk(NB - 1, c)

    nc.compile()
    return nc


def get_nc():
    if "nc" not in _CACHE:
        _CACHE["nc"] = _build()
    return _CACHE["nc"]


def make_in_maps(inputs):
    q = np.asarray(inputs["q"], np.float32)
    k = np.asarray(inputs["k"], np.float32)
    v = np.asarray(inputs["v"], np.float32)
    mask = np.asarray(inputs["mask"])
    Wq = np.asarray(inputs["Wq"], np.float32)
    Wk = np.asarray(inputs["Wk"], np.float32)
    Wv = np.asarray(inputs["Wv"], np.float32)
    Wo = np.asarray(inputs["Wo"], np.float32)

    per_batch = []
    for b in range(B):
        per_batch.append({
            "qT": np.ascontiguousarray(q[b].T).astype(nbf16),
            "kT": np.ascontiguousarray(k[b].T).astype(nbf16),
            "vT": np.ascontiguousarray(v[b].T).astype(nbf16),
            "maskT": np.ascontiguousarray(
                (~mask[b].astype(bool)).T).astype(nbf16),
        })
    per_group = []
    for g in range(4):
        sl = slice(g * GD, (g + 1) * GD)
        per_group.append({
            "wqT": np.ascontiguousarray(Wq[sl, :].T).astype(nbf16),
            "wkT": np.ascontiguousarray(Wk[sl, :].T).astype(nbf16),
            "wvT": np.ascontiguousarray(Wv[sl, :].T).astype(nbf16),
            "woT": np.ascontiguousarray(Wo[:, sl].T).astype(nbf16),
        })
    in_maps = []
    for c in range(N_CORES):
        b, g = c // 4, c % 4
        m = {}
        m.update(per_batch[b])
        m.update(per_group[g])
        in_maps.append(m)
    return in_maps


def assemble(results):
    out = np.zeros((B, S, D), np.float32)
    kp = np.empty((B, S, D), np.float32)
    vp = np.empty((B, S, D), np.float32)
    for c, res in enumerate(results):
        b, g = c // 4, c % 4
        kp[b][:, g * GD:(g + 1) * GD] = res["kpT_out"].T.astype(np.float32)
        vp[b][:, g * GD:(g + 1) * GD] = res["vp_out"].astype(np.float32)
        out[b] += res["outp_out"].astype(np.float32)
    return out, kp, vp


def run_cores(in_maps, trace=False, **kwargs):
    nc = get_nc()
    return bass_utils.run_bass_kernel_spmd(
        nc, in_maps, core_ids=list(range(N_CORES)), trace=trace, **kwargs
    )


def kernel(**inputs):
    in_maps = make_in_maps(inputs)
    res = run_cores(in_maps, trace=False)
    return assemble(res.results)
